# revision 1
# baseline (speedup 1.0000x reference)
"""Bidirectional Mamba block (BiT_MamSleep) on 8 TRN2 NeuronCores.

Sharding: core c handles (batch b = c//2, direction dir = c%2). Each core runs
the full pre-projection + its direction's selective scan in feature-major
layout (features on partitions, time on the free dim); the two cores of a pair
exchange their direction outputs with a pairwise AllReduce (the backward
core time-flips + masks before the exchange), then both compute the tail
(gate multiply, output projection, final LN) redundantly.

Selective scan: state pairs (d, s) are laid out on partitions (8 d-values x 16
states per 128-partition tile, 32 tiles for d_inner=256), time on the free dim,
computed with the DVE tensor_tensor_scan instruction (h = dA*h + dBu).
dt/dt*u are replicated 16x across states via DRAM-bounce broadcast DMAs;
exp(A*dt) runs on ScalarE with the per-partition A as the activation scale;
the dBu multiply runs on VectorE (scalar_tensor_tensor), the C multiply on
GpSimdE, and the 16-state contraction runs as a 0/1-mask matmul on TensorE.
"""
import sys

if '/opt/trn_rl_repo' not in sys.path:
    sys.path.insert(0, '/opt/trn_rl_repo')

import numpy as np

import concourse.bass as bass
import concourse.bacc as bacc
import concourse.tile as tile
from concourse import mybir
from concourse.bass_utils import run_bass_kernel_spmd

HID = 128
BATCH = 4
SEQ = 2048
D_STATE = 16
D_CONV = 4
D_INNER = 256
DT_RANK = 8

L = SEQ
C = HID
CW = 512           # matmul / PSUM chunk width
NCH = L // CW
PL = 512           # P3 scan piece width
NPC = L // PL
NK = 32            # (d,s) partition tiles: 256*16/128
f32 = mybir.dt.float32
mult = mybir.AluOpType.mult
add = mybir.AluOpType.add
sub = mybir.AluOpType.subtract
AF = mybir.ActivationFunctionType

_PROGRAM = None


def _declare(nc):
    dp = lambda name, shape: nc.declare_dram_parameter(name, list(shape), f32, isOutput=False)
    p = {}
    p['x'] = dp('x', (C, L))
    for n in ('wlmT', 'wlgT', 'wcT', 'loT'):
        p[n] = dp(n, (C, C))
    p['inwT'] = dp('inwT', (C, 2 * D_INNER))
    p['xpwT0'] = dp('xpwT0', (128, DT_RANK + 2 * D_STATE))
    p['xpwT1'] = dp('xpwT1', (128, DT_RANK + 2 * D_STATE))
    p['dtwT'] = dp('dtwT', (DT_RANK, D_INNER))
    p['outwT0'] = dp('outwT0', (128, C))
    p['outwT1'] = dp('outwT1', (128, C))
    p['convw'] = dp('convw', (128, 2 * D_CONV))   # halves side by side
    p['avecs'] = dp('avecs', (128, NK))
    p['m16big'] = dp('m16big', (128, 16 * 128))
    for n in ('conv_b', 'dt_b', 'dp_v'):
        p[n] = dp(n, (128, 2))                    # halves in columns
    for n in ('bias_lm', 'bias_lg', 'bias_c', 'lo_b', 'ln_g', 'ln_b', 'm_fwd', 'm_bwd'):
        p[n] = dp(n, (C, 1))
    p['y'] = nc.declare_dram_parameter('y', [C, L], f32, isOutput=True)
    return p


class B:
    """Builder state shared by the stage helpers."""


def _proj(b, lhsT, rhs, out, func, bias, out_cols=None, rows=C):
    """out[:, cs] = func(lhsT.T @ rhs[:, cs] + bias) per CW-chunk (PE + ACT)."""
    nc = b.nc
    for ci in range(NCH):
        cs = slice(ci * CW, (ci + 1) * CW)
        ocs = cs if out_cols is None else slice(out_cols + ci * CW, out_cols + (ci + 1) * CW)
        ps = b.ps.tile([rows, CW], f32, name='bank', tag='bank')
        nc.tensor.matmul(ps, lhsT, rhs[:, cs], start=True, stop=True)
        nc.scalar.activation(out[:, ocs], ps, func, bias=bias)


def _layernorm(b, x_sb, pref):
    """LayerNorm over the 128 channels per column: returns a gp tile holding
    (x - mean) * rsqrt(var + eps). Stats via ones-matmul; the mean/rstd rows
    are broadcast back across partitions with K=1 ones-row matmuls (PSUM)."""
    nc = b.nc
    rows = b.io.tile([128, L], f32, name='lnrows', tag='lnrows')
    ex = rows[0:1, :]
    rr_ = ex  # same base-0 row reused per chunk (mean then rstd)
    nrm0 = b.gp_tile()
    sq2 = b.gp_tile()
    out = b.gp_tile()
    for ci in range(NCH):
        cs = slice(ci * CW, (ci + 1) * CW)
        ps0 = b.ps.tile([1, CW], f32, name='bank', tag='bank')
        nc.tensor.matmul(ps0, b.ones_col, x_sb[:, cs], start=True, stop=True)
        nc.scalar.activation(ex[:, cs], ps0, AF.Identity, bias=0.0, scale=1.0 / C)
        psb = b.ps.tile([128, CW], f32, name='bank', tag='bank')
        nc.tensor.matmul(psb, b.ones_row, ex[:, cs], start=True, stop=True)
        nc.vector.scalar_tensor_tensor(nrm0[:, cs], x_sb[:, cs], 1.0, psb, mult, sub)
        nc.scalar.activation(sq2[:, cs], nrm0[:, cs], AF.Square)
        psv = b.ps.tile([1, CW], f32, name='bank', tag='bank')
        nc.tensor.matmul(psv, b.ones_col, sq2[:, cs], start=True, stop=True)
        nc.scalar.activation(rr_[:, cs], psv, AF.Ln, bias=b.eps_t[:, :], scale=1.0 / C)
        nc.scalar.activation(rr_[:, cs], rr_[:, cs], AF.Exp, bias=0.0, scale=-0.5)
        psr = b.ps.tile([128, CW], f32, name='bank', tag='bank')
        nc.tensor.matmul(psr, b.ones_row, rr_[:, cs], start=True, stop=True)
        nc.vector.scalar_tensor_tensor(out[:, cs], nrm0[:, cs], 1.0, psr, mult, mult)
    return out


def _build_body(nc, tc, p, ctx):
    b = B()
    b.nc = nc
    b.io = ctx.enter_context(tc.tile_pool(name='io', bufs=1))
    b.gp = ctx.enter_context(tc.tile_pool(name='gp', bufs=6))
    b.rot = ctx.enter_context(tc.tile_pool(name='rot', bufs=4))
    b.ps = ctx.enter_context(tc.tile_pool(name='ps', bufs=4, space='PSUM'))
    b.py = ctx.enter_context(tc.tile_pool(name='py', bufs=1, space='PSUM'))
    b.dram = ctx.enter_context(tc.tile_pool(name='drm', bufs=1, space='DRAM'))
    b.gp_tile = lambda: b.gp.tile([C, L], f32, name='g', tag='g')

    # ---- load weights/vectors ----
    W = {}
    for n, shape in (('wlmT', (C, C)), ('wlgT', (C, C)), ('wcT', (C, C)),
                     ('loT', (C, C)), ('inwT', (C, 2 * D_INNER)),
                     ('xpwT0', (128, 40)), ('xpwT1', (128, 40)),
                     ('dtwT', (8, 256)), ('outwT0', (128, C)), ('outwT1', (128, C)),
                     ('convw', (128, 8)), ('avecs', (128, NK)), ('m16big', (128, 16 * 128))):
        W[n] = b.io.tile(list(shape), f32, name=n, tag=n)
        nc.sync.dma_start(out=W[n], in_=p[n][:, :])
    V = {}
    for n in ('conv_b', 'dt_b', 'dp_v'):
        V[n] = b.io.tile([128, 2], f32, name=n, tag=n)
        nc.sync.dma_start(out=V[n], in_=p[n][:, :])
    for n in ('bias_lm', 'bias_lg', 'bias_c', 'lo_b', 'ln_g', 'ln_b', 'm_fwd', 'm_bwd'):
        V[n] = b.io.tile([C, 1], f32, name=n, tag=n)
        nc.sync.dma_start(out=V[n], in_=p[n][:, :])
    ones_col = b.io.tile([C, 1], f32, name='ones_col', tag='ones_col')
    nc.vector.memset(ones_col, 1.0)
    b.ones_col = ones_col
    eps_t = b.io.tile([1, 1], f32, name='lneps', tag='lneps')
    nc.vector.memset(eps_t, 1e-5)
    b.eps_t = eps_t
    ones_row = b.io.tile([1, 128], f32, name='ones_row', tag='ones_row')
    nc.vector.memset(ones_row, 1.0)
    b.ones_row = ones_row

    x = b.gp_tile()
    nc.sync.dma_start(out=x, in_=p['x'][:, :])

    # ---- P1: input layernorm over channels ----
    nrm = _layernorm(b, x, 'l1')

    # ---- P2: projections ----
    xmf_pre = b.gp_tile()
    _proj(b, W['wlmT'], nrm, xmf_pre, AF.Identity, V['bias_lm'][:, :])
    gate = b.io.tile([C, L], f32, name='gate', tag='gate')
    _proj(b, W['wlgT'], nrm, gate, AF.Silu, V['bias_lg'][:, :])
    xm = b.gp_tile()
    _proj(b, W['wcT'], xmf_pre, xm, AF.Silu, V['bias_c'][:, :])

    u_pad = []
    sz_t = []
    for h in range(2):
        up = b.io.tile([128, D_CONV - 1 + L], f32, name=f'upad{h}', tag=f'upad{h}')
        nc.vector.memset(up[:, 0:D_CONV - 1], 0.0)
        _proj(b, W['inwT'][:, 128 * h:128 * (h + 1)], xm, up, AF.Identity, 0.0,
              out_cols=D_CONV - 1)
        u_pad.append(up)
        szt = b.io.tile([128, L], f32, name=f'sz{h}', tag=f'sz{h}')
        _proj(b, W['inwT'][:, 256 + 128 * h:256 + 128 * (h + 1)], xm, szt, AF.Silu, 0.0)
        sz_t.append(szt)

    # causal depthwise conv (4 taps) + silu
    uc = []
    for h in range(2):
        cw = W['convw'][:, 4 * h:4 * (h + 1)]
        acc = b.gp_tile()
        nc.vector.tensor_scalar_mul(acc, u_pad[h][:, 3:3 + L], cw[:, 3:4])
        for kk in range(3):
            nc.vector.scalar_tensor_tensor(
                acc, u_pad[h][:, kk:kk + L], cw[:, kk:kk + 1], acc, mult, add)
        uct = b.io.tile([128, L], f32, name=f'uc{h}', tag=f'uc{h}')
        nc.scalar.activation(uct, acc, AF.Silu, bias=V['conv_b'][:, h:h + 1])
        uc.append(uct)

    # dbl = xp_w @ uc -> dtr(8), B(16), Cm(16)
    dbl_sb = b.io.tile([128, L], f32, name='dbl_sb', tag='dbl_sb')
    dtr = dbl_sb[0:8, :]
    b_sb = dbl_sb[32:48, :]
    c_sb = dbl_sb[64:80, :]
    for ci in range(NCH):
        cs = slice(ci * CW, (ci + 1) * CW)
        for dst, lo, hi in ((dtr, 0, 8), (b_sb, 8, 24), (c_sb, 24, 40)):
            ps_dbl = b.ps.tile([hi - lo, CW], f32, name='bank', tag='bank')
            nc.tensor.matmul(ps_dbl, W['xpwT0'][:, lo:hi], uc[0][:, cs],
                             start=True, stop=False)
            nc.tensor.matmul(ps_dbl, W['xpwT1'][:, lo:hi], uc[1][:, cs],
                             start=False, stop=True)
            nc.scalar.activation(dst[:, cs], ps_dbl, AF.Identity, bias=0.0)

    # dt = softplus(dt_w @ dtr + dt_b); dtu = dt * uc; stash both to DRAM
    dt_d = []
    dtu_d = []
    for h in range(2):
        # softplus(z) = ln(1 + exp(z)) -- no softplus entry in the ACT tables
        dtt = b.gp_tile()
        _proj(b, W['dtwT'][:, 128 * h:128 * (h + 1)], dtr, dtt, AF.Exp,
              V['dt_b'][:, h:h + 1], rows=128)
        nc.scalar.activation(dtt, dtt, AF.Ln, bias=1.0, scale=1.0)
        dtut = b.gp_tile()
        nc.vector.scalar_tensor_tensor(dtut, dtt, 1.0, uc[h], mult, mult)
        dd = b.dram.tile([128, L], f32, name=f'dtd{h}', tag=f'dtd{h}')
        ud = b.dram.tile([128, L], f32, name=f'dtud{h}', tag=f'dtud{h}')
        nc.sync.dma_start(out=dd, in_=dtt)
        nc.sync.dma_start(out=ud, in_=dtut)
        dt_d.append(dd)
        dtu_d.append(ud)

    # B_rep / C_rep : (16,L) replicated 8x across partitions
    b_rep = b.io.tile([128, L], f32, name='b_rep', tag='b_rep')
    c_rep = b.io.tile([128, L], f32, name='c_rep', tag='c_rep')
    for j in range(8):
        nc.sync.dma_start(out=b_rep[16 * j:16 * (j + 1), :], in_=b_sb[:, :])
        nc.sync.dma_start(out=c_rep[16 * j:16 * (j + 1), :], in_=c_sb[:, :])

    # ---- P3: selective scan over 32 (d,s)-tiles ----
    yz = []
    for h in range(2):
        psy = b.py.tile([128, L], f32, name='psy', tag='psy')
        for r in range(16):
            k = 16 * h + r
            ht_prev = None
            for pc in range(NPC):
                pcs = slice(pc * PL, (pc + 1) * PL)
                dtrep = b.rot.tile([128, PL], f32, name='dtrep', tag='dtrep')
                src = bass.AP(tensor=dt_d[h].tensor,
                              offset=dt_d[h].offset + 8 * r * L + pc * PL,
                              ap=[[L, 8], [0, 16], [1, PL]])
                nc.sync.dma_start(out=dtrep, in_=src)
                dturep = b.rot.tile([128, PL], f32, name='dturep', tag='dturep')
                src = bass.AP(tensor=dtu_d[h].tensor,
                              offset=dtu_d[h].offset + 8 * r * L + pc * PL,
                              ap=[[L, 8], [0, 16], [1, PL]])
                nc.sync.dma_start(out=dturep, in_=src)

                da = b.rot.tile([128, PL], f32, name='da', tag='da')
                nc.scalar.activation(da, dtrep, AF.Exp, bias=0.0,
                                     scale=W['avecs'][:, k:k + 1])
                dbu = b.rot.tile([128, PL], f32, name='dbu', tag='dbu')
                nc.vector.scalar_tensor_tensor(dbu, dturep, 1.0, b_rep[:, pcs],
                                               mult, mult)
                ht = b.rot.tile([128, PL], f32, name='ht', tag='ht')
                init = 0.0 if pc == 0 else ht_prev[:, PL - 1:PL]
                nc.vector.tensor_tensor_scan(ht, da, dbu, init, mult, add)
                ht_prev = ht
                for ci in range(PL // CW):
                    ccs = slice(ci * CW, (ci + 1) * CW)
                    ocs = slice(pc * PL + ci * CW, pc * PL + (ci + 1) * CW)
                    ycm = b.rot.tile([128, CW], f32, name='ycm', tag='ycm', bufs=4)
                    nc.gpsimd.tensor_mul(ycm, ht[:, ccs], c_rep[:, ocs])
                    nc.tensor.matmul(psy[:, ocs], W['m16big'][:, 128 * r:128 * (r + 1)],
                                     ycm, start=(r == 0), stop=(r == 15),
                                     skip_group_check=True)
        # y1 = uc*Dp + psy ; yz = y1 * silu(z)
        yzt = b.io.tile([128, D_CONV - 1 + L], f32, name=f'upad{h}', tag=f'upad{h}')
        nc.vector.scalar_tensor_tensor(
            yzt[:, 0:L], uc[h], V['dp_v'][:, h:h + 1], psy, mult, add)
        nc.vector.scalar_tensor_tensor(yzt[:, 0:L], yzt[:, 0:L], 1.0, sz_t[h],
                                       mult, mult)
        yz.append(yzt)

    # out projection: y_dir = out_w @ (y * silu(z))
    y_dir = b.gp_tile()
    for ci in range(NCH):
        cs = slice(ci * CW, (ci + 1) * CW)
        ps_o = b.ps.tile([C, CW], f32, name='bank', tag='bank')
        nc.tensor.matmul(ps_o, W['outwT0'], yz[0][:, cs], start=True, stop=False)
        nc.tensor.matmul(ps_o, W['outwT1'], yz[1][:, cs], start=False, stop=True)
        nc.scalar.activation(y_dir[:, cs], ps_o, AF.Identity, bias=0.0)

    # ---- P4: flip (backward dir), select, pairwise exchange ----
    y_flip = b.gp_tile()
    nc.vector.tensor_copy(y_flip, y_dir[:, ::-1])
    y_sel = b.gp_tile()
    nc.vector.tensor_scalar_mul(y_sel, y_dir, V['m_fwd'][:, :])
    nc.vector.scalar_tensor_tensor(y_sel, y_flip, V['m_bwd'][:, :], y_sel, mult, add)

    cc_in = b.dram.tile([C, L], f32, name='cc_in', tag='cc_in')
    cc_out = b.dram.tile([C, L], f32, name='cc_out', tag='cc_out')
    nc.sync.dma_start(out=cc_in, in_=y_sel)
    nc.gpsimd.collective_compute(
        'AllReduce', add,
        replica_groups=[[0, 1], [2, 3], [4, 5], [6, 7]],
        ins=[cc_in.opt()], outs=[cc_out.opt()])
    y_sum = b.gp_tile()
    nc.sync.dma_start(out=y_sum, in_=cc_out)

    # ---- P5: tail: gate multiply, lo projection, final LN ----
    g1 = b.gp_tile()
    nc.vector.scalar_tensor_tensor(g1, y_sum, 1.0, gate, mult, mult)
    t2 = b.gp_tile()
    _proj(b, W['loT'], g1, t2, AF.Identity, V['lo_b'][:, :])

    o1 = _layernorm(b, t2, 'l2')
    out_sb = b.gp_tile()
    nc.scalar.activation(out_sb, o1, AF.Identity, bias=V['ln_b'][:, :],
                         scale=V['ln_g'][:, :])
    nc.sync.dma_start(out=p['y'][:, :], in_=out_sb)


def _build_program():
    import contextlib
    nc = bacc.Bacc('TRN2', target_bir_lowering=False, debug=False, num_devices=8)
    p = _declare(nc)
    with tile.TileContext(nc) as tc:
        with contextlib.ExitStack() as ctx:
            _build_body(nc, tc, p, ctx)
    nc.compile()
    return nc


def _prep_core_inputs(inputs, bidx, d):
    g = lambda n: np.asarray(inputs[n], dtype=np.float32)
    x = g('x')
    ln_g = g('ln_g')
    ln_b = g('ln_b')
    pre = 'mf_' if d == 0 else 'mb_'
    P = lambda n: np.asarray(inputs[pre + n], dtype=np.float32)

    lm_w, lm_b = g('lm_w'), g('lm_b')
    lg_w, lg_b = g('lg_w'), g('lg_b')
    lo_w, lo_b = g('lo_w'), g('lo_b')
    if d == 0:
        wc, cb = g('cf_w'), g('cf_b')
    else:
        wc, cb = np.ascontiguousarray(g('cb_w')[:, ::-1]), g('cb_b')

    A = -np.exp(P('Alog'))                       # (256,16)
    avecs = np.ascontiguousarray(A.reshape(NK, 128).T)   # col k = flat[128k:128k+128]
    m16big = np.zeros((128, 16 * 128), np.float32)
    for r in range(16):
        m16big[np.arange(128), 128 * r + 8 * r + np.arange(128) // 16] = 1.0

    halves = lambda v: np.ascontiguousarray(
        np.stack([v[:128], v[128:]], axis=1).astype(np.float32))
    col = lambda v: np.ascontiguousarray(v.astype(np.float32).reshape(-1, 1))
    T = lambda w: np.ascontiguousarray(w.T.astype(np.float32))

    xpwT = np.ascontiguousarray(P('xp_w').T)     # (256,40)
    outwT = np.ascontiguousarray(P('out_w').T)   # (256,128)
    cwn = P('conv_w')                            # (256,4)
    convw = np.ascontiguousarray(np.concatenate([cwn[:128], cwn[128:]], axis=1))

    return {
        'x': np.ascontiguousarray(x[bidx]),
        'wlmT': T(lm_w * ln_g[None, :]),
        'wlgT': T(lg_w * ln_g[None, :]),
        'wcT': T(wc),
        'loT': T(lo_w),
        'inwT': T(P('in_w')),
        'xpwT0': np.ascontiguousarray(xpwT[:128]),
        'xpwT1': np.ascontiguousarray(xpwT[128:]),
        'dtwT': np.ascontiguousarray(P('dt_w').T),
        'outwT0': np.ascontiguousarray(outwT[:128]),
        'outwT1': np.ascontiguousarray(outwT[128:]),
        'convw': convw,
        'avecs': avecs,
        'm16big': m16big,
        'conv_b': halves(P('conv_b')),
        'dt_b': halves(P('dt_b')),
        'dp_v': halves(P('D')),
        'bias_lm': col(lm_w @ ln_b + lm_b),
        'bias_lg': col(lg_w @ ln_b + lg_b),
        'bias_c': col(cb),
        'lo_b': col(lo_b),
        'ln_g': col(ln_g),
        'ln_b': col(ln_b),
        'm_fwd': np.full((C, 1), 1.0 if d == 0 else 0.0, np.float32),
        'm_bwd': np.full((C, 1), 0.0 if d == 0 else 1.0, np.float32),
    }


def get_program():
    global _PROGRAM
    if _PROGRAM is None:
        _PROGRAM = _build_program()
    return _PROGRAM


def run(inputs, **run_kwargs):
    nc = get_program()
    in_maps = [_prep_core_inputs(inputs, c // 2, c % 2) for c in range(8)]
    res = run_bass_kernel_spmd(nc, in_maps, core_ids=list(range(8)), **run_kwargs)
    out = np.stack([res.results[2 * b]['y'] for b in range(BATCH)], axis=0)
    return out, res


def kernel(**inputs) -> np.ndarray:
    out, _ = run(inputs)
    return out.astype(np.float32)

